# revision 29
# baseline (speedup 1.0000x reference)
"""Multi-head attention (B=8, N=1024, C=768, H=12) on 8 Trainium2 NeuronCores.

Sharding: data-parallel over batch — one batch element per core, no collectives.

Per-core dataflow (all layouts chosen so NO on-chip transposes are needed):
  - Host pre-transposes x and the weights into contraction-on-partition layouts.
  - Q^T,K^T computed in [o, n] layout (o on partitions), V in [n, o] layout with a
    65-stride per-head interleave whose 65th column is set to 1.0 (ones-augmented
    V) so the attn@V matmul also produces the softmax row-sums for free.
  - S^T[m, n] = K^T.T @ Q^T per head (contraction over d=64 on partitions).
  - P^T = exp(0.125 * S^T) on ScalarE (no max-subtraction: logits ~ N(0,1)).
  - O'[d, n] (+rowsum row) = V'aug.T @ P^T, accumulated over m-tiles in PSUM.
  - normalize: broadcast 1/rowsum over partitions (SWDGE stride-0 DMA) and
    multiply; store O' in [c, n] layout (two heads stacked per 128-partition tile).
  - final = O'.T @ proj_w^T + bias in [n, c'] layout, DMA'd out.
All matmuls run as float32r (full-rate single-pass) with fp32 PSUM accumulation.
"""

import numpy as np

_STATE = {}

B, N, C = 8, 1024, 768
H, D = 12, 64
KT = 6           # contraction tiles of 128 over C
P = 128
NT = N // P      # 8 n-tiles
PAIRS = H // 2   # 6 head pairs
VW = H * (D + 1)  # 780: ones-augmented per-head V width


def _patch_tile_drain():
    """This walrus build rejects >1 sem wait on a CTRL (Drain) instruction.

    TileContext's exit puts one wait per outstanding semaphore on the final SP
    Drain; redistribute them across single-wait NOPs preceding the drain.
    """
    import bass_rust
    import concourse.tile as tile
    from concourse.vector_clock import ScopedClock

    if getattr(tile.TileContext, "_ant_drain_patched", False):
        return

    SyncInfo = bass_rust.SyncInfo

    def _drain_and_barrier(self, tick_clock, wait_clock):
        nc = self.nc
        probe = nc.sync.nop(nofuse=True)
        wait_clock.add_sem_waits(
            probe.ins, ScopedClock({None: tick_clock.global_clock})
        )
        si = probe.ins.sync_info
        waits = list(si.on_wait or []) if si is not None else []
        updates = list(si.on_update or []) if si is not None else []
        if len(waits) > 1:
            probe.ins.sync_info = SyncInfo(on_wait=waits[:1], on_update=updates)
            for w in waits[1:]:
                extra = nc.sync.nop(nofuse=True)
                extra.ins.sync_info = SyncInfo(on_wait=[w], on_update=[])
        nc.sync.drain()

        nc.all_engine_barrier()
        assert self.sems is not None
        popped = nc._tile_sem_poison_stack.pop()
        assert popped is self._sem_poison
        nc.clear_and_free_semaphores(list(self.sems.allocated().values()))
        nc.all_engine_barrier()

    tile.TileContext._drain_and_barrier = _drain_and_barrier
    tile.TileContext._ant_drain_patched = True


def _split_multi_waits(nc):
    """This walrus build allows at most ONE sem wait per instruction.

    Tile's wait assignment routinely puts several; hoist all but the last onto
    single-wait NOPs inserted immediately before the instruction on the same
    engine (engines execute block instructions in order, so semantics are
    unchanged).
    """
    from concourse import mybir

    for fn in nc.m.functions:
        for bb in fn.blocks:
            out, changed = [], False
            for inst in bb.instructions:
                si = inst.sync_info
                waits = list(si.on_wait) if (si is not None and si.on_wait) else []
                if len(waits) > 1:
                    changed = True
                    for w in waits[:-1]:
                        nop = mybir.InstNoOp(
                            name=f"I-ws{nc.next_id()}",
                            engine=inst.engine,
                            bass_nofuse=True,
                            sync_info=mybir.SyncInfo(on_wait=[w], on_update=[]),
                        )
                        nc.register_instruction(nop)
                        out.append(nop)
                    inst.sync_info = mybir.SyncInfo(
                        on_wait=[waits[-1]], on_update=list(si.on_update or [])
                    )
                out.append(inst)
            if changed:
                bb.instructions = out


def _build_nc(trace_sim=False):
    from contextlib import ExitStack

    import concourse.bass as bass
    import concourse.tile as tile
    from concourse import mybir

    _patch_tile_drain()

    f32 = mybir.dt.float32
    f32r = mybir.dt.float32r

    nc = bass.Bass("TRN2", target_bir_lowering=False, debug=False, num_devices=1)

    xT = nc.dram_tensor("xT", [KT, P, N], f32r, kind="ExternalInput").ap()
    wqk = nc.dram_tensor("wqk", [PAIRS, P, KT * 256], f32r, kind="ExternalInput").ap()
    wv = nc.dram_tensor("wv", [P, KT, VW], f32r, kind="ExternalInput").ap()
    pT = nc.dram_tensor("pT", [P, KT, C], f32r, kind="ExternalInput").ap()
    bias = nc.dram_tensor("bias", [P, C], f32, kind="ExternalInput").ap()
    ones = nc.dram_tensor("ones", [P, H], f32r, kind="ExternalInput").ap()
    pt5hi = nc.dram_tensor("pt5hi", [D, C], f32r, kind="ExternalInput").ap()
    y = nc.dram_tensor("y", [N, C], f32, kind="ExternalOutput").ap()

    Exp = mybir.ActivationFunctionType.Exp
    SCALE = float(D) ** -0.5

    with tile.TileContext(nc, trace_sim=trace_sim) as tc, ExitStack() as ctx:
        kilo = ctx.enter_context(tc.tile_pool(name="kilo", bufs=6))      # xT
        op_ = ctx.enter_context(tc.tile_pool(name="op", bufs=6))        # O'
        qkp = ctx.enter_context(tc.tile_pool(name="qk", bufs=4))
        bigp = ctx.enter_context(tc.tile_pool(name="big", bufs=2))       # wv, pT
        wqkp = ctx.enter_context(tc.tile_pool(name="wqk", bufs=2))
        vp = ctx.enter_context(tc.tile_pool(name="v", bufs=8))
        ptp = ctx.enter_context(tc.tile_pool(name="pt", bufs=4))
        rbp = ctx.enter_context(tc.tile_pool(name="rb", bufs=3))
        tbp = ctx.enter_context(tc.tile_pool(name="tb", bufs=1))
        outp = ctx.enter_context(tc.tile_pool(name="out", bufs=8))
        onep = ctx.enter_context(tc.tile_pool(name="one", bufs=1))
        drp = ctx.enter_context(tc.tile_pool(name="dr", bufs=2, space="DRAM"))
        ps = ctx.enter_context(tc.tile_pool(name="ps", bufs=4, space="PSUM"))

        # warm the ACT exp table set while input DMAs run (the first real exp
        # otherwise pays the ~2.7us ACT_TABLE_LOAD on the critical path)
        warm = onep.tile([1, 4], f32)
        nc.vector.memset(warm[:], 0.0)
        warm2 = onep.tile([1, 4], f32)
        nc.scalar.activation(warm2[:], warm[:], Exp)

        # ---- load constants / inputs ----
        xs = []
        for k in range(KT):
            t = kilo.tile([P, N], f32r, tag="kilo")
            eng = nc.sync if k % 2 == 0 else nc.gpsimd
            eng.dma_start(t[:, 0:512], xT[k][:, 0:512])
            xs.append(t)
        for k in range(KT):
            eng = nc.sync if k % 2 == 0 else nc.gpsimd
            eng.dma_start(xs[k][:, 512:1024], xT[k][:, 512:1024])

        qt_sb, kt_sb = [], []
        wq_tiles = {}

        def prefetch_wq(t):
            if t not in wq_tiles:
                wq_t = wqkp.tile([P, KT * 256], f32r, tag="wqk", name=f"wq_{t}")
                nc.sync.dma_start(wq_t[:], wqk[t])
                wq_tiles[t] = wq_t

        def emit_qk_one(t, which, store):
            prefetch_wq(t)
            wq_t = wq_tiles[t]
            slot = ps.tile([P, 1024], f32, tag="ps")
            for ns in range(2):
                dst = slot[:, ns * 512 : (ns + 1) * 512]
                for k in range(KT):
                    nc.tensor.matmul(
                        dst,
                        wq_t[:, k * 256 + which * P : k * 256 + (which + 1) * P],
                        xs[k][:, ns * 512 : (ns + 1) * 512],
                        start=(k == 0),
                        stop=(k == KT - 1),
                    )
            qk_t = qkp.tile([P, N], f32r, tag="qk")
            nc.vector.tensor_copy(qk_t[:], slot[:, 0:1024])
            store.append(qk_t)

        # QK of pair 0 first so attention can start early
        prefetch_wq(0)
        emit_qk_one(0, 0, qt_sb)
        emit_qk_one(0, 1, kt_sb)

        wv_sb = bigp.tile([P, KT, VW], f32r, tag="big")
        nc.gpsimd.dma_start(wv_sb[:], wv[:])

        bias_sb = onep.tile([P, C], f32)
        nc.gpsimd.dma_start(bias_sb[:], bias[:])
        pt5hi_sb = onep.tile([D, C], f32r)
        nc.gpsimd.dma_start(pt5hi_sb[:], pt5hi[:])
        tb5p = ctx.enter_context(tc.tile_pool(name="tb5", bufs=1))
        tb5_holder = []

        # ---- V' in [n, 780] layout (ones-augmented heads), emitted just-in-time ----
        v_sb = []

        def emit_v(nt):
            slot = ps.tile([P, 1024], f32, tag="ps")
            for half, (c0, w) in enumerate(((0, 390), (390, 390))):
                dst = slot[:, half * 512 : half * 512 + w]
                for k in range(KT):
                    nc.tensor.matmul(
                        dst,
                        xs[k][:, nt * P : (nt + 1) * P],
                        wv_sb[:, k, c0 : c0 + w],
                        start=(k == 0),
                        stop=(k == KT - 1),
                    )
            vt = vp.tile([P, VW], f32r, tag="v")
            nc.vector.tensor_copy(vt[:, 0:390], slot[:, 0:390])
            nc.vector.tensor_copy(vt[:, 390:780], slot[:, 512:902])
            ones_ap = vt.rearrange("p (h w) -> p h w", w=D + 1)[:, :, D]
            nc.sync.dma_start(ones_ap, ones[:])
            v_sb.append(vt)

        emit_v(0)

        # ---- attention, one head at a time; V'/QK of upcoming work interleaved ----
        o_sb = []
        pt_w = None
        for t in range(PAIRS):
            if t == 1:
                pt_w = bigp.tile([P, KT, C], f32r, tag="big")
                nc.gpsimd.dma_start(pt_w[:], pT[:])
            ot = op_.tile([P, N], f32r, tag="op")
            for head in range(2):
                hb = head * D
                h = 2 * t + head
                o_slot = ps.tile([P, 1024], f32, tag="ps")
                for j in range(NT):
                    s_slot = ps.tile([P, 1024], f32, tag="ps")
                    for ns in range(2):
                        nc.tensor.matmul(
                            s_slot[:, ns * 512 : (ns + 1) * 512],
                            kt_sb[t][hb : hb + D, j * P : (j + 1) * P],
                            qt_sb[t][hb : hb + D, ns * 512 : (ns + 1) * 512],
                            start=True,
                            stop=True,
                        )
                    pt_t = ptp.tile([P, 1024], f32r, tag="pt")
                    nc.scalar.activation(pt_t[:], s_slot[:], Exp, scale=SCALE)
                    for ns in range(2):
                        nc.tensor.matmul(
                            o_slot[0 : D + 1, ns * 512 : (ns + 1) * 512],
                            v_sb[j][:, h * (D + 1) : (h + 1) * (D + 1)],
                            pt_t[:, ns * 512 : (ns + 1) * 512],
                            start=(j == 0),
                            stop=(j == NT - 1),
                            skip_group_check=True,
                        )
                    if t == 0 and head == 0 and j + 1 < NT:
                        emit_v(j + 1)
                    if head == 0 and t + 1 < PAIRS and j == 6:
                        prefetch_wq(t + 1)
                    if head == 1 and t + 1 < PAIRS:
                        if j == 2:
                            emit_qk_one(t + 1, 0, qt_sb)
                        elif j == 5:
                            emit_qk_one(t + 1, 1, kt_sb)
                # normalize: O'[0:64] / rowsum (row 64); heads stacked in ot
                rb = rbp.tile([P, N], f32, tag="rb")
                nc.vector.tensor_copy(rb[D : D + 1, :], o_slot[D : D + 1, :])
                scratch = drp.tile([1, N], f32, tag="dr")
                nc.sync.dma_start(scratch[0:1, :], rb[D : D + 1, :])
                bcast_src = bass.AP(
                    tensor=scratch.tensor,
                    offset=scratch.offset,
                    ap=[[0, D]] + [list(dd) for dd in scratch[0:1, :].ap[1:]],
                )
                nc.gpsimd.dma_start(out=rb[0:D, :], in_=bcast_src)
                nc.vector.reciprocal(rb[0:D, :], rb[0:D, :])
                if head == 0:
                    nc.vector.tensor_mul(ot[0:D, :], o_slot[0:D, :], rb[0:D, :])
                elif t == PAIRS - 1:
                    tb5 = tb5p.tile([D, N], f32r, tag="tb5")
                    nc.vector.tensor_mul(tb5[:], o_slot[0:D, :], rb[0:D, :])
                    tb5_holder.append(tb5)
                else:
                    tb = tbp.tile([D, N], f32r, tag="tb")
                    nc.vector.tensor_mul(tb[:], o_slot[0:D, :], rb[0:D, :])
                    nc.sync.dma_start(ot[D:P, :], tb[:])
            o_sb.append(ot)

        # ---- projection + bias ----
        # k=5 depends on the last pair's normalize; accumulate k=0..4 (+bias)
        # into SBUF per n-tile first (frees the PSUM unit immediately, so all
        # eight partials overlap the last pair), then only the two half-K k=5
        # matmuls and a final add remain on the exposed tail.
        acc_sb = {}

        def proj_partial(nt):
            slot = ps.tile([P, 1024], f32, tag="ps", name=f"proj_{nt}")
            for k in range(KT - 1):
                for c0, w in ((0, 512), (512, 256)):
                    nc.tensor.matmul(
                        slot[:, c0 : c0 + w],
                        o_sb[k][:, nt * P : (nt + 1) * P],
                        pt_w[:, k, c0 : c0 + w],
                        start=(k == 0),
                        stop=(k == KT - 2),
                        skip_group_check=True,
                    )
            acc = outp.tile([P, C], f32, tag="out", name=f"acc_{nt}")
            nc.vector.tensor_add(acc[:], slot[:, 0:C], bias_sb[:])
            acc_sb[nt] = acc

        def proj_finish(nt):
            slot = ps.tile([P, 1024], f32, tag="ps", name=f"projf_{nt}")
            k = KT - 1
            for c0, w in ((0, 512), (512, 256)):
                nc.tensor.matmul(
                    slot[:, c0 : c0 + w],
                    o_sb[k][0:D, nt * P : (nt + 1) * P],
                    pt_w[0:D, k, c0 : c0 + w],
                    start=True,
                    stop=False,
                    skip_group_check=True,
                )
            for c0, w in ((0, 512), (512, 256)):
                nc.tensor.matmul(
                    slot[:, c0 : c0 + w],
                    tb5_holder[0][:, nt * P : (nt + 1) * P],
                    pt5hi_sb[:, c0 : c0 + w],
                    start=False,
                    stop=True,
                    skip_group_check=True,
                )
            acc = acc_sb.pop(nt)
            nc.vector.tensor_add(acc[:], acc[:], slot[:, 0:C])
            nc.sync.dma_start(y[nt * P : (nt + 1) * P, :], acc[:])

        for nt in range(NT):
            proj_partial(nt)
        for nt in range(NT):
            proj_finish(nt)

    _split_multi_waits(nc)
    return nc


def _prep_shared(qkv_w, proj_w, proj_b):
    f = np.float32
    wq = qkv_w[0:C].astype(f)          # [o, c]
    wk = qkv_w[C : 2 * C].astype(f)
    wv_ = qkv_w[2 * C : 3 * C].astype(f)
    wqT, wkT, wvT = wq.T.copy(), wk.T.copy(), wv_.T.copy()  # [c, o]

    wqk = np.zeros((PAIRS, P, KT, 256), f)
    for t in range(PAIRS):
        for k in range(KT):
            wqk[t, :, k, 0:P] = wqT[k * P : (k + 1) * P, t * P : (t + 1) * P]
            wqk[t, :, k, P:256] = wkT[k * P : (k + 1) * P, t * P : (t + 1) * P]
    wqk = wqk.reshape(PAIRS, P, KT * 256)

    wvh = np.zeros((P, KT, H, D + 1), f)
    for k in range(KT):
        wvh[:, k, :, 0:D] = wvT[k * P : (k + 1) * P].reshape(P, H, D)
    wvh = wvh.reshape(P, KT, VW)

    pTh = proj_w.T.astype(f).reshape(KT, P, C).transpose(1, 0, 2).copy()
    pt5hi = np.ascontiguousarray(proj_w.T.astype(f)[C - D : C, :])
    bias_h = np.ascontiguousarray(np.broadcast_to(proj_b.astype(f), (P, C)))
    return wqk, wvh, pTh, bias_h, pt5hi


def kernel(x, qkv_w, proj_w, proj_b):
    from concourse.bass_utils import run_bass_kernel_spmd

    x = np.asarray(x, np.float32)
    wqk, wvh, pTh, bias_h, pt5hi = _prep_shared(
        np.asarray(qkv_w), np.asarray(proj_w), np.asarray(proj_b)
    )

    if "nc" not in _STATE:
        _STATE["nc"] = _build_nc()
    nc = _STATE["nc"]

    in_maps = []
    for b in range(B):
        xTb = np.ascontiguousarray(x[b].T).reshape(KT, P, N)
        in_maps.append(
            {"xT": xTb, "wqk": wqk, "wv": wvh, "pT": pTh, "bias": bias_h,
             "ones": np.ones((P, H), np.float32), "pt5hi": pt5hi}
        )

    res = run_bass_kernel_spmd(nc, in_maps, core_ids=list(range(B)))
    return np.stack([res.results[b]["y"] for b in range(B)], axis=0)
